# revision 11
# baseline (speedup 1.0000x reference)
"""Feedforward SNN (Linear -> LIF) x2 kernel for Trainium2, 8-core data parallel.

Per-core plan (B sharded 8 ways, BL=32 samples/core):
  - Host pre-transposes operands once (cheap numpy): xT[d, (t,b)] per core,
    W1T[d, h1], W2T[h1, h2], all split Dekker-style into fp16 (high, low)
    pairs stored interleaved. The device runs ONLY matmuls and LIF scans.
  - The PE datapath is natively FP22 (11-bit mantissa): fp16 operands upcast
    exactly, so a (high, low) fp16 pair carries ~21 bits. Layer-1 currents
    need full fp32-class precision (spike-threshold crossings cascade hard),
    so mm1 = 3 cross terms (xh*wh + xl*wh + xh*wl), residual ~2^-21.
  - Layer-2's moving operand is the {0,1} spike tensor (exact in fp16);
    mm2 = 2 terms (W2h + W2l) @ spk, residual ~2^-22.
  - All matmul passes run at 1 cycle/row (fp16 = bf16 speed), weight loads
    hidden by fast-weight-load + background weight buffer.
  - Layer-1 currents for ALL timesteps of a t-block: Cur1[h1, (t,b)] =
    W1 @ x^T (x does not depend on recurrent state).
  - LIF scans over t on [128, HC*32] tiles (partition = h % 128, free =
    (hchunk, b)); fused scalar_tensor_tensor DVE ops.
  - Software-pipelined: mm1(nb+1) is emitted before mm2(nb) so the PE fills
    the scan1(nb) latency; PE phases are chained with order-only deps.
"""

import os
import sys

import numpy as np

for _p in ("/opt/trn_rl_repo", "/root/.axon_site/_ro/trn_rl_repo"):
    if os.path.isdir(_p) and _p not in sys.path:
        sys.path.insert(0, _p)

import concourse.bass as bass  # noqa: E402
import concourse.mybir as mybir  # noqa: E402
import concourse.tile as tile  # noqa: E402
from concourse import bacc  # noqa: E402
from concourse.bass_utils import run_bass_kernel_spmd  # noqa: E402
from concourse.masks import make_identity  # noqa: E402
from concourse.tile_rust import add_dep_helper  # noqa: E402

F32 = mybir.dt.float32
F16 = mybir.dt.float16
BF16 = mybir.dt.bfloat16
ALU = mybir.AluOpType
AF = mybir.ActivationFunctionType

BETA = 0.9
THR = 1.0

B_FULL, T_FULL, D_FULL, H1_FULL, H2_FULL = 256, 64, 1024, 2048, 2048
N_CORES = 8
BL = B_FULL // N_CORES  # 32


def build_snn(T=T_FULL, D=D_FULL, H1=H1_FULL, H2=H2_FULL, T_NB=16):
    """Build the single-core Bass program (identical across the 8 cores)."""
    P = 128
    KC1 = D // P
    HC1 = H1 // P
    HC2 = H2 // P
    NNB = T // T_NB
    SUB = min(4, T_NB)
    NSUB = T_NB // SUB
    MCQ = min(4, HC2)
    HCQ = min(4, HC1)
    NB32 = T_NB * 32          # matmul free dim per t-block

    assert T % T_NB == 0 and T_NB % SUB == 0
    assert HC2 % MCQ == 0 and HC1 % HCQ == 0

    nc = bacc.Bacc("TRN2", target_bir_lowering=False, debug=False)

    # x variants: 0 = xh, 1 = xh * 2^-11 (de-scales W1's scaled low term),
    # 2 = xl.  W1 pair: (wh, wl * 2^11) -- the low is host-scaled back into
    # fp16 normal range (raw wl ~1e-5 would hit the subnormal floor).
    xt_d = nc.dram_tensor("xT3", [D, 3, T * BL], F16, kind="ExternalInput")
    w1t_d = nc.dram_tensor("W1Thl", [D, 2, H1], F16, kind="ExternalInput")
    b1_d = nc.dram_tensor("b1", [H1], F32, kind="ExternalInput")
    w2t_d = nc.dram_tensor("W2Thl", [H1, 2, H2], F16, kind="ExternalInput")
    b2_d = nc.dram_tensor("b2", [H2], F32, kind="ExternalInput")

    spk2_d = nc.dram_tensor("spk2", [BL, H2], F32, kind="ExternalOutput")
    mem1_d = nc.dram_tensor("mem1", [BL, H1], F32, kind="ExternalOutput")
    mem2_d = nc.dram_tensor("mem2", [BL, H2], F32, kind="ExternalOutput")

    with tile.TileContext(nc) as tc:
        from contextlib import ExitStack
        ctx = ExitStack()
        with ctx:
            const = ctx.enter_context(tc.tile_pool(name="const", bufs=1))
            xtp = ctx.enter_context(tc.tile_pool(name="xtp", bufs=2))
            w1tp = ctx.enter_context(tc.tile_pool(name="w1tp", bufs=8))
            w2tp = ctx.enter_context(tc.tile_pool(name="w2tp", bufs=8))
            curp = ctx.enter_context(tc.tile_pool(name="curp", bufs=6))
            spk1p = ctx.enter_context(tc.tile_pool(name="spk1p", bufs=1))
            statep = ctx.enter_context(tc.tile_pool(name="statep", bufs=2))
            negzp = ctx.enter_context(tc.tile_pool(name="negzp", bufs=1))
            outp = ctx.enter_context(tc.tile_pool(name="outp", bufs=4))
            tpsum = ctx.enter_context(
                tc.tile_pool(name="tpsum", bufs=2, space="PSUM"))
            mpsum = ctx.enter_context(
                tc.tile_pool(name="mpsum", bufs=6, space="PSUM"))

            ident = const.tile([P, P], F32, name="ident")
            make_identity(nc, ident)

            # PE phase chaining (order-only deps): keeps the mm phases
            # from interleaving in the PE stream.
            pe_phases = []

            class _Ph:
                def __init__(self):
                    self.insts = []

                def add(self, bi):
                    self.insts.append(bi.ins)

            b1s = const.tile([P, HC1], F32, name="b1s")
            nc.gpsimd.dma_start(
                b1s[:], b1_d.ap().rearrange("(c p) -> p c", p=P))
            b2s = const.tile([P, HC2], F32, name="b2s")
            nc.gpsimd.dma_start(
                b2s[:], b2_d.ap().rearrange("(c p) -> p c", p=P))

            # ---------------- PE warmup (HAM ramp) --------------------------
            wub = const.tile([P, 512], BF16, name="wub")
            nc.vector.memset(wub[:], 0.0)
            wuw = const.tile([P, P], BF16, name="wuw")
            nc.vector.memset(wuw[:], 0.0)
            ph = _Ph()
            pe_phases.append(ph)
            wups = mpsum.tile([P, 512], F32, tag="mm", name="wups")
            for i in range(24):
                ph.add(nc.tensor.matmul(wups[:], wuw[:], wub[:],
                                        start=(i == 0), stop=(i == 23)))

            # ---------------- initial LIF state ----------------------------
            mem1_cur = statep.tile([P, HC1, 32], F32, tag="mem1",
                                   name="mem1_0")
            nc.vector.memset(mem1_cur[:], 0.0)
            mem2_cur = statep.tile([P, HC2, 32], F32, tag="mem2",
                                   name="mem2_0")
            nc.vector.memset(mem2_cur[:], 0.0)
            spk2_fin = const.tile([P, HC2, 32], F32, name="spk2_fin")

            # ---------------- outputs helper --------------------------------
            def emit_out(state, nch, out_d):
                ph = _Ph()
                pe_phases.append(ph)
                for hc in range(nch):
                    ps = tpsum.tile([32, P], F32, tag="tp", name="ops")
                    ph.add(nc.tensor.transpose(ps[:], state[:, hc, :],
                                               ident[:]))
                    sb = outp.tile([32, P], F32, tag="osb", name="osb")
                    nc.scalar.activation(sb[:], ps[:], AF.Copy)
                    nc.sync.dma_start(
                        out_d.ap()[:, hc * P:(hc + 1) * P], sb[:])

            # ---------------- per-block emitters ----------------------------
            def x_and_mm1(nb):
                """xT load + matmul1 (3-term fp16 Dekker) for block nb."""
                ph = _Ph()
                pe_phases.append(ph)
                t0 = nb * T_NB
                xq = nc.sync if nb == 0 else nc.gpsimd
                xt = xtp.tile([P, KC1, 3, NB32], F16, tag="xt", name="xt")
                for kc in range(KC1):
                    xq.dma_start(
                        xt[:, kc, :, :],
                        xt_d.ap()[kc * P:(kc + 1) * P, :,
                                  t0 * 32:(t0 + T_NB) * 32])

                cur1_subs = [curp.tile([P, SUB, HC1, 32], F32, tag="cur1",
                                       bufs=6, name="cur1")
                             for _ in range(NSUB)]
                for hq in range(HC1 // HCQ):
                    pss = [mpsum.tile([P, NB32], F32, tag="mm", name="mm1ps")
                           for _ in range(HCQ)]
                    for kc in range(KC1):
                        w1tt = w1tp.tile([P, 2, HCQ * P], F16, tag="w1t",
                                         name="w1tt")
                        dq = nc.sync if kc % 2 == 0 else nc.scalar
                        dq.dma_start(
                            w1tt[:],
                            w1t_d.ap()[kc * P:(kc + 1) * P, :,
                                       hq * HCQ * P:(hq + 1) * HCQ * P])
                        xh = xt[:, kc, 0, :]
                        xds = xt[:, kc, 1, :]
                        xl = xt[:, kc, 2, :]
                        for i in range(HCQ):
                            wh = w1tt[:, 0, i * P:(i + 1) * P]
                            wl = w1tt[:, 1, i * P:(i + 1) * P]
                            ph.add(nc.tensor.matmul(
                                pss[i][:], wh, xh,
                                start=(kc == 0), stop=False))
                            ph.add(nc.tensor.matmul(
                                pss[i][:], wl, xds,
                                start=False, stop=False))
                            ph.add(nc.tensor.matmul(
                                pss[i][:], wh, xl,
                                start=False, stop=(kc == KC1 - 1)))
                    for i in range(HCQ):
                        hc = hq * HCQ + i
                        psv = pss[i].rearrange("p (t b) -> p t b", b=32)
                        for s in range(NSUB):
                            nc.scalar.activation(
                                cur1_subs[s][:, :, hc, :],
                                psv[:, s * SUB:(s + 1) * SUB, :],
                                AF.Identity, bias=b1s[:, hc:hc + 1])
                return cur1_subs

            # ---------------- main t-block pipeline -------------------------
            # software pipelining: mm1(nb+1) is emitted BEFORE mm2(nb) so the
            # PE stream (priority = program order) fills the scan1(nb)
            # latency with mm1(nb+1) instead of stalling on spk1.
            cur1_next = x_and_mm1(0)
            for nb in range(NNB):
                t0 = nb * T_NB
                cur1_subs = cur1_next
                if nb + 1 < NNB:
                    cur1_next = x_and_mm1(nb + 1)

                # -- scan1 (T_NB steps); spikes (fp16) into spk1[(kc,t,b)] ---
                spk1 = spk1p.tile([P, HC1, NB32], F16, tag="spk1",
                                  name="spk1")
                for tr in range(T_NB):
                    cur_t = cur1_subs[tr // SUB][:, tr % SUB]  # [P, HC1, 32]
                    negz = negzp.tile([P, HC1, 32], F32, tag="negz1",
                                      name="negz1")
                    nc.vector.scalar_tensor_tensor(
                        negz[:], mem1_cur[:], THR, cur_t,
                        ALU.is_gt, ALU.subtract)
                    mem1_new = statep.tile([P, HC1, 32], F32, tag="mem1",
                                           name="mem1")
                    nc.vector.scalar_tensor_tensor(
                        mem1_new[:], mem1_cur[:], BETA, negz[:],
                        ALU.mult, ALU.subtract)
                    mem1_cur = mem1_new
                    # spike of step t thresholds the POST-update membrane
                    nc.vector.tensor_scalar(
                        spk1[:, :, tr * 32:(tr + 1) * 32], mem1_cur[:],
                        THR, None, ALU.is_gt)

                if nb == NNB - 1:
                    emit_out(mem1_cur, HC1, mem1_d)

                # -- matmul2 (2x fp16): cur2[(t,mc,b)] = W2 @ spk1^T + b2 ----
                ph = _Ph()
                pe_phases.append(ph)
                cur2_subs = [curp.tile([P, SUB, HC2, 32], F32, tag="cur2",
                                       bufs=5, name="cur2")
                             for _ in range(NSUB)]
                for mq in range(HC2 // MCQ):
                    pss = [mpsum.tile([P, NB32], F32, tag="mm", name="mm2ps")
                           for _ in range(MCQ)]
                    for kc in range(HC1):
                        wt = w2tp.tile([P, 2, MCQ * P], F16, tag="w2t",
                                       name="w2t")
                        dq = nc.sync if kc % 2 == 0 else nc.scalar
                        dq.dma_start(
                            wt[:],
                            w2t_d.ap()[kc * P:(kc + 1) * P, :,
                                       mq * MCQ * P:(mq + 1) * MCQ * P])
                        rhs = spk1[:, kc, :]
                        for i in range(MCQ):
                            ph.add(nc.tensor.matmul(
                                pss[i][:], wt[:, 0, i * P:(i + 1) * P], rhs,
                                start=(kc == 0), stop=False))
                            ph.add(nc.tensor.matmul(
                                pss[i][:], wt[:, 1, i * P:(i + 1) * P], rhs,
                                start=False, stop=(kc == HC1 - 1)))
                    for i in range(MCQ):
                        mc = mq * MCQ + i
                        psv = pss[i].rearrange("p (t b) -> p t b", b=32)
                        for s in range(NSUB):
                            nc.scalar.activation(
                                cur2_subs[s][:, :, mc, :],
                                psv[:, s * SUB:(s + 1) * SUB, :],
                                AF.Identity, bias=b2s[:, mc:mc + 1])

                # -- scan2 (T_NB steps) --------------------------------------
                for tr in range(T_NB):
                    t = t0 + tr
                    cur_t = cur2_subs[tr // SUB][:, tr % SUB]
                    negz = negzp.tile([P, HC2, 32], F32, tag="negz2",
                                      name="negz2")
                    nc.vector.scalar_tensor_tensor(
                        negz[:], mem2_cur[:], THR, cur_t,
                        ALU.is_gt, ALU.subtract)
                    mem2_new = statep.tile([P, HC2, 32], F32, tag="mem2",
                                           name="mem2")
                    nc.vector.scalar_tensor_tensor(
                        mem2_new[:], mem2_cur[:], BETA, negz[:],
                        ALU.mult, ALU.subtract)
                    mem2_cur = mem2_new
                    if t == T - 1:
                        nc.vector.tensor_scalar(
                            spk2_fin[:], mem2_cur[:], THR, None, ALU.is_gt)

            # ---------------- remaining outputs -----------------------------
            emit_out(mem2_cur, HC2, mem2_d)
            emit_out(spk2_fin, HC2, spk2_d)

            # chain consecutive PE phases: every inst of phase b ordered
            # after the last inst of phase a (order-only deps)
            for a, b in zip(pe_phases, pe_phases[1:]):
                if a.insts and b.insts:
                    for bi in b.insts:
                        add_dep_helper(bi, a.insts[-1], sync=False,
                                       reason="PE phase ordering")

    nc.compile()
    return nc


_NC_CACHE = {}


def _get_nc():
    if "full" not in _NC_CACHE:
        _NC_CACHE["full"] = build_snn()
    return _NC_CACHE["full"]


def _hl_pair(a, scale_low=None):
    """Split fp32 array into (high, low) fp16 pair stacked on axis 1.
    With scale_low, the low term is multiplied by it (keeps tiny lows out
    of the fp16 subnormal floor; de-scaled via the moving operand)."""
    h = a.astype(np.float16)
    low = a - h.astype(np.float32)
    if scale_low:
        low = low * np.float32(scale_low)
    return np.ascontiguousarray(np.stack([h, low.astype(np.float16)],
                                         axis=1))


def prep_inputs(x, W1, b1, W2, b2):
    """Host-side prep: shard x over cores (transposed to [d, (t,b)]),
    transpose W1/W2, split everything into fp16 (high, low) pairs."""
    x = np.asarray(x, np.float32)
    W1 = np.asarray(W1, np.float32)
    b1 = np.ascontiguousarray(np.asarray(b1, np.float32))
    W2 = np.asarray(W2, np.float32)
    b2 = np.ascontiguousarray(np.asarray(b2, np.float32))
    B, T, D = x.shape

    W1Thl = _hl_pair(W1.T, scale_low=2048.0)            # [D, 2, H1]
    W2Thl = _hl_pair(W2.T)                              # [H1, 2, H2]

    bl = B // N_CORES
    in_maps = []
    for c in range(N_CORES):
        xc = x[c * bl:(c + 1) * bl]                     # [bl, T, D]
        xT = np.ascontiguousarray(
            xc.transpose(2, 1, 0).reshape(D, T * bl))   # [d, (t,b)] t-major
        xh = xT.astype(np.float16)
        xds = (xh.astype(np.float32) * np.float32(1.0 / 2048.0)).astype(
            np.float16)
        xl = (xT - xh.astype(np.float32)).astype(np.float16)
        xT3 = np.ascontiguousarray(np.stack([xh, xds, xl], axis=1))
        in_maps.append({
            "xT3": xT3, "W1Thl": W1Thl, "b1": b1,
            "W2Thl": W2Thl, "b2": b2,
        })
    return in_maps


def kernel(x, W1, b1, W2, b2):
    """Full-input entry point: shards B across 8 NeuronCores, returns full
    (spk2, mem1, mem2) exactly like reference()."""
    nc = _get_nc()
    in_maps = prep_inputs(x, W1, b1, W2, b2)
    res = run_bass_kernel_spmd(nc, in_maps, core_ids=list(range(N_CORES)))
    spk2 = np.concatenate([res.results[c]["spk2"] for c in range(N_CORES)], 0)
    mem1 = np.concatenate([res.results[c]["mem1"] for c in range(N_CORES)], 0)
    mem2 = np.concatenate([res.results[c]["mem2"] for c in range(N_CORES)], 0)
    return spk2, mem1, mem2


# revision 17
# speedup vs baseline: 1.2353x; 1.2353x over previous
"""Feedforward SNN (Linear -> LIF) x2 kernel for Trainium2, 8-core data parallel.

Per-core plan (B sharded 8 ways, BL=32 samples/core):
  - Host pre-transposes operands once (cheap numpy): xT[d, (t,b)] per core,
    W1T[d, h1], W2T[h1, h2], all split Dekker-style into fp16 (high, low)
    pairs stored interleaved. The device runs ONLY matmuls and LIF scans.
  - The PE datapath is natively FP22 (11-bit mantissa): fp16 operands upcast
    exactly, so a (high, low) fp16 pair carries ~21 bits. Layer-1 currents
    need full fp32-class precision (spike-threshold crossings cascade hard),
    so mm1 = 3 cross terms (xh*wh + xl*wh + xh*wl), residual ~2^-21.
  - Layer-2's moving operand is the {0,1} spike tensor (exact in fp16);
    mm2 = 2 terms (W2h + W2l) @ spk, residual ~2^-22.
  - All matmul passes run at 1 cycle/row (fp16 = bf16 speed), weight loads
    hidden by fast-weight-load + background weight buffer.
  - Layer-1 currents for ALL timesteps of a t-block: Cur1[h1, (t,b)] =
    W1 @ x^T (x does not depend on recurrent state).
  - LIF scans over t on [128, HC*32] tiles (partition = h % 128, free =
    (hchunk, b)); fused scalar_tensor_tensor DVE ops.
  - Software-pipelined: mm1(nb+1) is emitted before mm2(nb) so the PE fills
    the scan1(nb) latency; PE phases are chained with order-only deps.
"""

import os
import sys

import numpy as np

for _p in ("/opt/trn_rl_repo", "/root/.axon_site/_ro/trn_rl_repo"):
    if os.path.isdir(_p) and _p not in sys.path:
        sys.path.insert(0, _p)

import ml_dtypes  # noqa: E402

import concourse.bass as bass  # noqa: E402
import concourse.mybir as mybir  # noqa: E402
import concourse.tile as tile  # noqa: E402
from concourse import bacc  # noqa: E402
from concourse.bass_utils import run_bass_kernel_spmd  # noqa: E402
from concourse.masks import make_identity  # noqa: E402
from concourse.tile_rust import add_dep_helper  # noqa: E402

F32 = mybir.dt.float32
F16 = mybir.dt.float16
BF16 = mybir.dt.bfloat16
ALU = mybir.AluOpType
AF = mybir.ActivationFunctionType

BETA = 0.9
THR = 1.0

B_FULL, T_FULL, D_FULL, H1_FULL, H2_FULL = 256, 64, 1024, 2048, 2048
N_CORES = 8
BL = B_FULL // N_CORES  # 32


def build_snn(T=T_FULL, D=D_FULL, H1=H1_FULL, H2=H2_FULL, T_NB=16):
    """Build the single-core Bass program (identical across the 8 cores)."""
    P = 128
    KC1 = D // P
    HC1 = H1 // P
    HC2 = H2 // P
    NNB = T // T_NB
    SUB = min(4, T_NB)
    NSUB = T_NB // SUB
    MCQ = min(4, HC2)
    HCQ = min(4, HC1)
    NB32 = T_NB * 32          # matmul free dim per t-block

    assert T % T_NB == 0 and T_NB % SUB == 0
    assert HC2 % MCQ == 0 and HC1 % HCQ == 0

    nc = bacc.Bacc("TRN2", target_bir_lowering=False, debug=False)

    # x variants: 0 = xh, 1 = xh * 2^-11 (de-scales W1's scaled low term),
    # 2 = xl.  W1 pair: (wh, wl * 2^11) -- the low is host-scaled back into
    # fp16 normal range (raw wl ~1e-5 would hit the subnormal floor).
    xt_d = nc.dram_tensor("xT3", [D, 3, T * BL], F16, kind="ExternalInput")
    w1t_d = nc.dram_tensor("W1Thl", [D, 2, H1], F16, kind="ExternalInput")
    b1_d = nc.dram_tensor("b1", [H1], F32, kind="ExternalInput")
    w2t_d = nc.dram_tensor("W2Thl", [H1, 2, H2], BF16,
                           kind="ExternalInput")
    b2_d = nc.dram_tensor("b2", [H2], F32, kind="ExternalInput")

    spk2_d = nc.dram_tensor("spk2", [BL, H2], F32, kind="ExternalOutput")
    mem1_d = nc.dram_tensor("mem1", [BL, H1], F32, kind="ExternalOutput")
    mem2_d = nc.dram_tensor("mem2", [BL, H2], F32, kind="ExternalOutput")

    with tile.TileContext(nc) as tc:
        from contextlib import ExitStack
        ctx = ExitStack()
        with ctx:
            const = ctx.enter_context(tc.tile_pool(name="const", bufs=1))
            xtp = ctx.enter_context(tc.tile_pool(name="xtp", bufs=2))
            w1tp = ctx.enter_context(tc.tile_pool(name="w1tp", bufs=6))
            w2tp = ctx.enter_context(tc.tile_pool(name="w2tp", bufs=4))
            curp = ctx.enter_context(tc.tile_pool(name="curp", bufs=6))
            spk1p = ctx.enter_context(tc.tile_pool(name="spk1p", bufs=2))
            statep = ctx.enter_context(tc.tile_pool(name="statep", bufs=2))
            negzp = ctx.enter_context(tc.tile_pool(name="negzp", bufs=1))
            outp = ctx.enter_context(tc.tile_pool(name="outp", bufs=2))
            tpsum = ctx.enter_context(
                tc.tile_pool(name="tpsum", bufs=2, space="PSUM"))
            mpsum = ctx.enter_context(
                tc.tile_pool(name="mpsum", bufs=6, space="PSUM"))

            ident = const.tile([P, P], F32, name="ident")
            make_identity(nc, ident)

            # PE phase chaining (order-only deps): keeps the mm phases
            # from interleaving in the PE stream.
            pe_phases = []

            class _Ph:
                def __init__(self):
                    self.insts = []

                def add(self, bi):
                    self.insts.append(bi.ins)

            b1s = const.tile([P, HC1], F32, name="b1s")
            nc.gpsimd.dma_start(
                b1s[:], b1_d.ap().rearrange("(c p) -> p c", p=P))
            b2s = const.tile([P, HC2], F32, name="b2s")
            nc.gpsimd.dma_start(
                b2s[:], b2_d.ap().rearrange("(c p) -> p c", p=P))

            # ---------------- PE warmup (HAM ramp) --------------------------
            wub = const.tile([P, 512], BF16, name="wub")
            nc.vector.memset(wub[:], 0.0)
            wuw = const.tile([P, P], BF16, name="wuw")
            nc.vector.memset(wuw[:], 0.0)
            ph = _Ph()
            pe_phases.append(ph)
            wups = mpsum.tile([P, 512], F32, tag="mm", name="wups")
            for i in range(24):
                ph.add(nc.tensor.matmul(wups[:], wuw[:], wub[:],
                                        start=(i == 0), stop=(i == 23)))

            # ---------------- initial LIF state ----------------------------
            mem1_cur = statep.tile([P, HC1, 32], F32, tag="mem1",
                                   name="mem1_0")
            nc.vector.memset(mem1_cur[:], 0.0)
            mem2_cur = statep.tile([P, HC2, 32], F32, tag="mem2",
                                   name="mem2_0")
            nc.vector.memset(mem2_cur[:], 0.0)
            spk2_fin = const.tile([P, HC2, 32], F32, name="spk2_fin")

            # ---------------- outputs helper --------------------------------
            def emit_out(state, nch, out_d):
                ph = _Ph()
                pe_phases.append(ph)
                for hc in range(nch):
                    ps = tpsum.tile([32, P], F32, tag="tp", name="ops")
                    ph.add(nc.tensor.transpose(ps[:], state[:, hc, :],
                                               ident[:]))
                    sb = outp.tile([32, P], F32, tag="osb", name="osb")
                    nc.scalar.activation(sb[:], ps[:], AF.Copy)
                    nc.sync.dma_start(
                        out_d.ap()[:, hc * P:(hc + 1) * P], sb[:])

            # ---------------- per-block emitters ----------------------------
            def x_and_mm1(nb):
                """xT load + matmul1 (3-term fp16 Dekker) for block nb."""
                ph = _Ph()
                pe_phases.append(ph)
                t0 = nb * T_NB
                xq = nc.sync if nb == 0 else nc.gpsimd
                xt = xtp.tile([P, KC1, 3, NB32], F16, tag="xt", name="xt")
                for kc in range(KC1):
                    xq.dma_start(
                        xt[:, kc, :, :],
                        xt_d.ap()[kc * P:(kc + 1) * P, :,
                                  t0 * 32:(t0 + T_NB) * 32])

                cur1_subs = [curp.tile([P, SUB, HC1, 32], F32, tag="cur1",
                                       bufs=6, name="cur1")
                             for _ in range(NSUB)]
                for hq in range(HC1 // HCQ):
                    pss = [mpsum.tile([P, NB32], F32, tag="mm", name="mm1ps")
                           for _ in range(HCQ)]
                    for kc in range(KC1):
                        w1tt = w1tp.tile([P, 2, HCQ * P], F16, tag="w1t",
                                         name="w1tt")
                        dq = (nc.scalar if nb == 0 else
                              (nc.sync if kc % 2 == 0 else nc.scalar))
                        dq.dma_start(
                            w1tt[:],
                            w1t_d.ap()[kc * P:(kc + 1) * P, :,
                                       hq * HCQ * P:(hq + 1) * HCQ * P])
                        xh = xt[:, kc, 0, :]
                        xds = xt[:, kc, 1, :]
                        xl = xt[:, kc, 2, :]
                        for i in range(HCQ):
                            wh = w1tt[:, 0, i * P:(i + 1) * P]
                            wl = w1tt[:, 1, i * P:(i + 1) * P]
                            ph.add(nc.tensor.matmul(
                                pss[i][:], wh, xh,
                                start=(kc == 0), stop=False))
                            ph.add(nc.tensor.matmul(
                                pss[i][:], wl, xds,
                                start=False, stop=False))
                            ph.add(nc.tensor.matmul(
                                pss[i][:], wh, xl,
                                start=False, stop=(kc == KC1 - 1)))
                    for i in range(HCQ):
                        hc = hq * HCQ + i
                        psv = pss[i].rearrange("p (t b) -> p t b", b=32)
                        for s in range(NSUB):
                            nc.scalar.activation(
                                cur1_subs[s][:, :, hc, :],
                                psv[:, s * SUB:(s + 1) * SUB, :],
                                AF.Identity, bias=b1s[:, hc:hc + 1])
                return cur1_subs

            # ---------------- scan1 emitter ---------------------------------
            def scan1_block(nb, cur1_subs):
                """LIF-1 scan for block nb -> spk1 tile (bf16)."""
                nonlocal mem1_cur
                spk1 = spk1p.tile([P, HC1, NB32], BF16, tag="spk1",
                                  name="spk1")
                for tr in range(T_NB):
                    cur_t = cur1_subs[tr // SUB][:, tr % SUB]  # [P, HC1, 32]
                    negz = negzp.tile([P, HC1, 32], F32, tag="negz",
                                      name="negz1")
                    nc.vector.scalar_tensor_tensor(
                        negz[:], mem1_cur[:], THR, cur_t,
                        ALU.is_gt, ALU.subtract)
                    mem1_new = statep.tile([P, HC1, 32], F32, tag="mem1",
                                           name="mem1")
                    nc.vector.scalar_tensor_tensor(
                        mem1_new[:], mem1_cur[:], BETA, negz[:],
                        ALU.mult, ALU.subtract)
                    mem1_cur = mem1_new
                    # spike of step t thresholds the POST-update membrane
                    nc.vector.tensor_scalar(
                        spk1[:, :, tr * 32:(tr + 1) * 32], mem1_cur[:],
                        THR, None, ALU.is_gt)
                return spk1

            # ---------------- main t-block pipeline -------------------------
            # software pipelining: mm1(nb+1) AND scan1(nb+1) are emitted
            # BEFORE mm2(nb)/scan2(nb).  PE order: mm1(0) mm1(1) mm2(0)
            # mm1(2) mm2(1) ... (program order = priority), and the DVE FIFO
            # runs scan1(nb+1) before scan2(nb), so mm2(nb+1) never waits on
            # a scan chain (spk1 is double-buffered to allow this overlap).
            cur1_next = x_and_mm1(0)
            spk1_cur = scan1_block(0, cur1_next)
            for nb in range(NNB):
                t0 = nb * T_NB
                if nb + 1 < NNB:
                    cur1_next = x_and_mm1(nb + 1)
                    spk1_next = scan1_block(nb + 1, cur1_next)
                spk1 = spk1_cur

                # -- matmul2 (2x bf16): cur2[(t,mc,b)] = W2 @ spk1^T + b2 ----
                ph = _Ph()
                pe_phases.append(ph)
                cur2_subs = [curp.tile([P, SUB, HC2, 32], F32, tag="cur2",
                                       bufs=5, name="cur2")
                             for _ in range(NSUB)]
                for mq in range(HC2 // MCQ):
                    pss = [mpsum.tile([P, NB32], F32, tag="mm", name="mm2ps")
                           for _ in range(MCQ)]
                    for kc in range(HC1):
                        wt = w2tp.tile([P, 2, MCQ * P], BF16, tag="w2t",
                                       name="w2t")
                        dq = nc.sync if kc % 2 == 0 else nc.scalar
                        dq.dma_start(
                            wt[:],
                            w2t_d.ap()[kc * P:(kc + 1) * P, :,
                                       mq * MCQ * P:(mq + 1) * MCQ * P])
                        rhs = spk1[:, kc, :]
                        for i in range(MCQ):
                            ph.add(nc.tensor.matmul(
                                pss[i][:], wt[:, 0, i * P:(i + 1) * P], rhs,
                                start=(kc == 0), stop=False))
                            ph.add(nc.tensor.matmul(
                                pss[i][:], wt[:, 1, i * P:(i + 1) * P], rhs,
                                start=False, stop=(kc == HC1 - 1)))
                    for i in range(MCQ):
                        mc = mq * MCQ + i
                        psv = pss[i].rearrange("p (t b) -> p t b", b=32)
                        for s in range(NSUB):
                            nc.scalar.activation(
                                cur2_subs[s][:, :, mc, :],
                                psv[:, s * SUB:(s + 1) * SUB, :],
                                AF.Identity, bias=b2s[:, mc:mc + 1])

                # -- scan2 (T_NB steps) --------------------------------------
                for tr in range(T_NB):
                    t = t0 + tr
                    cur_t = cur2_subs[tr // SUB][:, tr % SUB]
                    negz = negzp.tile([P, HC2, 32], F32, tag="negz",
                                      name="negz2")
                    nc.vector.scalar_tensor_tensor(
                        negz[:], mem2_cur[:], THR, cur_t,
                        ALU.is_gt, ALU.subtract)
                    mem2_new = statep.tile([P, HC2, 32], F32, tag="mem2",
                                           name="mem2")
                    nc.vector.scalar_tensor_tensor(
                        mem2_new[:], mem2_cur[:], BETA, negz[:],
                        ALU.mult, ALU.subtract)
                    mem2_cur = mem2_new
                    if t == T - 1:
                        nc.vector.tensor_scalar(
                            spk2_fin[:], mem2_cur[:], THR, None, ALU.is_gt)

                if nb + 1 < NNB:
                    spk1_cur = spk1_next

            # ---------------- remaining outputs -----------------------------
            emit_out(mem1_cur, HC1, mem1_d)
            emit_out(mem2_cur, HC2, mem2_d)
            emit_out(spk2_fin, HC2, spk2_d)

            # chain consecutive PE phases: every inst of phase b ordered
            # after the last inst of phase a (order-only deps)
            for a, b in zip(pe_phases, pe_phases[1:]):
                if a.insts and b.insts:
                    for bi in b.insts:
                        add_dep_helper(bi, a.insts[-1], sync=False,
                                       reason="PE phase ordering")

    nc.compile()
    return nc


_NC_CACHE = {}


def _get_nc():
    if "full" not in _NC_CACHE:
        _NC_CACHE["full"] = build_snn()
    return _NC_CACHE["full"]


def _hl_pair(a, scale_low=None, dtype=np.float16):
    """Split fp32 array into (high, low) pairs stacked on axis 1.
    With scale_low, the low term is multiplied by it (keeps tiny lows out
    of the fp16 subnormal floor; de-scaled via the moving operand)."""
    h = a.astype(dtype)
    low = a - h.astype(np.float32)
    if scale_low:
        low = low * np.float32(scale_low)
    return np.ascontiguousarray(np.stack([h, low.astype(dtype)], axis=1))


def prep_inputs(x, W1, b1, W2, b2):
    """Host-side prep: shard x over cores (transposed to [d, (t,b)]),
    transpose W1/W2, split everything into fp16 (high, low) pairs."""
    x = np.asarray(x, np.float32)
    W1 = np.asarray(W1, np.float32)
    b1 = np.ascontiguousarray(np.asarray(b1, np.float32))
    W2 = np.asarray(W2, np.float32)
    b2 = np.ascontiguousarray(np.asarray(b2, np.float32))
    B, T, D = x.shape

    W1Thl = _hl_pair(W1.T, scale_low=2048.0)            # [D, 2, H1]
    W2Thl = _hl_pair(W2.T, dtype=ml_dtypes.bfloat16)    # [H1, 2, H2]

    bl = B // N_CORES
    in_maps = []
    for c in range(N_CORES):
        xc = x[c * bl:(c + 1) * bl]                     # [bl, T, D]
        xT = np.ascontiguousarray(
            xc.transpose(2, 1, 0).reshape(D, T * bl))   # [d, (t,b)] t-major
        xh = xT.astype(np.float16)
        xds = (xh.astype(np.float32) * np.float32(1.0 / 2048.0)).astype(
            np.float16)
        xl = (xT - xh.astype(np.float32)).astype(np.float16)
        xT3 = np.ascontiguousarray(np.stack([xh, xds, xl], axis=1))
        in_maps.append({
            "xT3": xT3, "W1Thl": W1Thl, "b1": b1,
            "W2Thl": W2Thl, "b2": b2,
        })
    return in_maps


def kernel(x, W1, b1, W2, b2):
    """Full-input entry point: shards B across 8 NeuronCores, returns full
    (spk2, mem1, mem2) exactly like reference()."""
    nc = _get_nc()
    in_maps = prep_inputs(x, W1, b1, W2, b2)
    res = run_bass_kernel_spmd(nc, in_maps, core_ids=list(range(N_CORES)))
    spk2 = np.concatenate([res.results[c]["spk2"] for c in range(N_CORES)], 0)
    mem1 = np.concatenate([res.results[c]["mem1"] for c in range(N_CORES)], 0)
    mem2 = np.concatenate([res.results[c]["mem2"] for c in range(N_CORES)], 0)
    return spk2, mem1, mem2


# revision 23
# speedup vs baseline: 1.2457x; 1.0084x over previous
"""Feedforward SNN (Linear -> LIF) x2 kernel for Trainium2, 8-core data parallel.

Per-core plan (B sharded 8 ways, BL=32 samples/core):
  - Host pre-transposes operands once (cheap numpy): xT[d, (t,b)] per core,
    W1T[d, h1], W2T[h1, h2], all split Dekker-style into fp16 (high, low)
    pairs stored interleaved. The device runs ONLY matmuls and LIF scans.
  - The PE datapath is natively FP22 (11-bit mantissa): fp16 operands upcast
    exactly, so a (high, low) fp16 pair carries ~21 bits. Layer-1 currents
    need full fp32-class precision (spike-threshold crossings cascade hard),
    so mm1 = 3 cross terms (xh*wh + xl*wh + xh*wl), residual ~2^-21.
  - Layer-2's moving operand is the {0,1} spike tensor (exact in fp16);
    mm2 = 2 terms (W2h + W2l) @ spk, residual ~2^-22.
  - All matmul passes run at 1 cycle/row (fp16 = bf16 speed), weight loads
    hidden by fast-weight-load + background weight buffer.
  - Layer-1 currents for ALL timesteps of a t-block: Cur1[h1, (t,b)] =
    W1 @ x^T (x does not depend on recurrent state).
  - LIF scans over t on [128, HC*32] tiles (partition = h % 128, free =
    (hchunk, b)); fused scalar_tensor_tensor DVE ops.
  - Software-pipelined: mm1(nb+1) is emitted before mm2(nb) so the PE fills
    the scan1(nb) latency; PE phases are chained with order-only deps.
"""

import os
import sys

import numpy as np

for _p in ("/opt/trn_rl_repo", "/root/.axon_site/_ro/trn_rl_repo"):
    if os.path.isdir(_p) and _p not in sys.path:
        sys.path.insert(0, _p)

import ml_dtypes  # noqa: E402

import concourse.bass as bass  # noqa: E402
import concourse.mybir as mybir  # noqa: E402
import concourse.tile as tile  # noqa: E402
from concourse import bacc  # noqa: E402
from concourse.bass_utils import run_bass_kernel_spmd  # noqa: E402
from concourse.masks import make_identity  # noqa: E402
from concourse.tile_rust import add_dep_helper  # noqa: E402

F32 = mybir.dt.float32
F16 = mybir.dt.float16
BF16 = mybir.dt.bfloat16
ALU = mybir.AluOpType
AF = mybir.ActivationFunctionType

BETA = 0.9
THR = 1.0

B_FULL, T_FULL, D_FULL, H1_FULL, H2_FULL = 256, 64, 1024, 2048, 2048
N_CORES = 8
BL = B_FULL // N_CORES  # 32


def build_snn(T=T_FULL, D=D_FULL, H1=H1_FULL, H2=H2_FULL, T_NB=16):
    """Build the single-core Bass program (identical across the 8 cores)."""
    P = 128
    KC1 = D // P
    HC1 = H1 // P
    HC2 = H2 // P
    NNB = T // T_NB
    SUB = min(4, T_NB)
    NSUB = T_NB // SUB
    MCQ = min(4, HC2)
    HCQ = min(4, HC1)
    NB32 = T_NB * 32          # matmul free dim per t-block

    assert T % T_NB == 0 and T_NB % SUB == 0
    assert HC2 % MCQ == 0 and HC1 % HCQ == 0

    nc = bacc.Bacc("TRN2", target_bir_lowering=False, debug=False)

    # x variants: 0 = xh, 1 = xh * 2^-11 (de-scales W1's scaled low term),
    # 2 = xl.  W1 pair: (wh, wl * 2^11) -- the low is host-scaled back into
    # fp16 normal range (raw wl ~1e-5 would hit the subnormal floor).
    xt_d = nc.dram_tensor("xT3", [D, 3, T * BL], F16, kind="ExternalInput")
    w1t_d = nc.dram_tensor("W1Thl", [D, 2, H1], F16, kind="ExternalInput")
    b1_d = nc.dram_tensor("b1", [H1], F32, kind="ExternalInput")
    w2t_d = nc.dram_tensor("W2Thl", [H1, 2, H2], BF16,
                           kind="ExternalInput")
    b2_d = nc.dram_tensor("b2", [H2], F32, kind="ExternalInput")

    spk2_d = nc.dram_tensor("spk2", [BL, H2], F32, kind="ExternalOutput")
    mem1_d = nc.dram_tensor("mem1", [BL, H1], F32, kind="ExternalOutput")
    mem2_d = nc.dram_tensor("mem2", [BL, H2], F32, kind="ExternalOutput")

    with tile.TileContext(nc) as tc:
        from contextlib import ExitStack
        ctx = ExitStack()
        with ctx:
            const = ctx.enter_context(tc.tile_pool(name="const", bufs=1))
            xtp = ctx.enter_context(tc.tile_pool(name="xtp", bufs=2))
            w1tp = ctx.enter_context(tc.tile_pool(name="w1tp", bufs=6))
            w2tp = ctx.enter_context(tc.tile_pool(name="w2tp", bufs=4))
            curp = ctx.enter_context(tc.tile_pool(name="curp", bufs=6))
            spk1p = ctx.enter_context(tc.tile_pool(name="spk1p", bufs=2))
            statep = ctx.enter_context(tc.tile_pool(name="statep", bufs=2))
            negzp = ctx.enter_context(tc.tile_pool(name="negzp", bufs=1))
            outp = ctx.enter_context(tc.tile_pool(name="outp", bufs=2))
            tpsum = ctx.enter_context(
                tc.tile_pool(name="tpsum", bufs=2, space="PSUM"))
            mpsum = ctx.enter_context(
                tc.tile_pool(name="mpsum", bufs=6, space="PSUM"))

            ident = const.tile([P, P], F32, name="ident")
            make_identity(nc, ident)

            # PE phase chaining (order-only deps): keeps the mm phases
            # from interleaving in the PE stream.
            pe_phases = []

            class _Ph:
                def __init__(self):
                    self.insts = []

                def add(self, bi):
                    self.insts.append(bi.ins)

            b1s = const.tile([P, HC1], F32, name="b1s")
            b2s = const.tile([P, HC2], F32, name="b2s")

            # ---------------- PE warmup (HAM ramp) --------------------------
            wub = const.tile([P, 512], BF16, name="wub")
            nc.vector.memset(wub[:], 0.0)
            wuw = const.tile([P, P], BF16, name="wuw")
            nc.vector.memset(wuw[:], 0.0)
            ph = _Ph()
            pe_phases.append(ph)
            wups = mpsum.tile([P, 512], F32, tag="mm", name="wups")
            for i in range(18):
                ph.add(nc.tensor.matmul(wups[:], wuw[:], wub[:],
                                        start=(i == 0), stop=(i == 17)))

            # ---------------- initial LIF state ----------------------------
            mem1_cur = statep.tile([P, HC1, 32], F32, tag="mem1",
                                   name="mem1_0")
            nc.vector.memset(mem1_cur[:], 0.0)
            mem2_cur = statep.tile([P, HC2, 32], F32, tag="mem2",
                                   name="mem2_0")
            nc.vector.memset(mem2_cur[:], 0.0)
            spk2_fin = const.tile([P, HC2, 32], F32, name="spk2_fin")

            # ---------------- outputs helper --------------------------------
            def emit_out(state, nch, out_d):
                # transpose 4 h-chunks at once: [128, (4hc, 32b)] -> psum
                # [(4hc, 32b), 128], then one DMA scatters to the dram rows
                # via a (hc b) h -> b (hc h) access pattern.
                ph = _Ph()
                pe_phases.append(ph)
                for hq in range(nch // 4):
                    ps = tpsum.tile([P, P], F32, tag="tp", name="ops")
                    ph.add(nc.tensor.transpose(
                        ps[:], state[:, hq * 4:(hq + 1) * 4, :], ident[:]))
                    sb = outp.tile([P, P], F32, tag="osb", name="osb")
                    nc.scalar.activation(sb[:], ps[:], AF.Copy)
                    # psum rows are (hc, b); walk the dram side in the same
                    # (hc, b, h) order (sizes may differ from src dims --
                    # dma only requires equal totals)
                    nc.sync.dma_start(
                        out_d.ap()[:, hq * 512:(hq + 1) * 512].rearrange(
                            "b (hc h) -> hc b h", hc=4), sb[:])

            # ---------------- per-block emitters ----------------------------
            def x_and_mm1(nb):
                """xT load + matmul1 (3-term fp16 Dekker) for block nb."""
                ph = _Ph()
                pe_phases.append(ph)
                t0 = nb * T_NB
                xq = nc.gpsimd
                xt = xtp.tile([P, KC1, 3, NB32], F16, tag="xt", name="xt")
                for kc in range(KC1):
                    xq.dma_start(
                        xt[:, kc, :, :],
                        xt_d.ap()[kc * P:(kc + 1) * P, :,
                                  t0 * 32:(t0 + T_NB) * 32])
                if nb == 0:
                    # bias gathers ride gpsimd behind block 0's x tiles so
                    # they never delay the first weight/x arrivals
                    nc.gpsimd.dma_start(
                        b1s[:], b1_d.ap().rearrange("(c p) -> p c", p=P))
                    nc.gpsimd.dma_start(
                        b2s[:], b2_d.ap().rearrange("(c p) -> p c", p=P))

                cur1_subs = [curp.tile([P, SUB, HC1, 32], F32, tag="cur1",
                                       bufs=6, name="cur1")
                             for _ in range(NSUB)]
                for hq in range(HC1 // HCQ):
                    pss = [mpsum.tile([P, NB32], F32, tag="mm", name="mm1ps")
                           for _ in range(HCQ)]
                    for kc in range(KC1):
                        w1tt = w1tp.tile([P, 2, HCQ * P], F16, tag="w1t",
                                         name="w1tt")
                        dq = nc.sync if kc % 2 == 0 else nc.scalar
                        dq.dma_start(
                            w1tt[:],
                            w1t_d.ap()[kc * P:(kc + 1) * P, :,
                                       hq * HCQ * P:(hq + 1) * HCQ * P])
                        xh = xt[:, kc, 0, :]
                        xds = xt[:, kc, 1, :]
                        xl = xt[:, kc, 2, :]
                        for i in range(HCQ):
                            wh = w1tt[:, 0, i * P:(i + 1) * P]
                            wl = w1tt[:, 1, i * P:(i + 1) * P]
                            ph.add(nc.tensor.matmul(
                                pss[i][:], wh, xh,
                                start=(kc == 0), stop=False))
                            ph.add(nc.tensor.matmul(
                                pss[i][:], wl, xds,
                                start=False, stop=False))
                            ph.add(nc.tensor.matmul(
                                pss[i][:], wh, xl,
                                start=False, stop=(kc == KC1 - 1)))
                    for i in range(HCQ):
                        hc = hq * HCQ + i
                        psv = pss[i].rearrange("p (t b) -> p t b", b=32)
                        for s in range(NSUB):
                            nc.scalar.activation(
                                cur1_subs[s][:, :, hc, :],
                                psv[:, s * SUB:(s + 1) * SUB, :],
                                AF.Identity, bias=b1s[:, hc:hc + 1])
                return cur1_subs

            # ---------------- scan1 emitter ---------------------------------
            def scan1_block(nb, cur1_subs):
                """LIF-1 scan for block nb -> spk1 tile (bf16)."""
                nonlocal mem1_cur
                spk1 = spk1p.tile([P, HC1, NB32], BF16, tag="spk1",
                                  name="spk1")
                for tr in range(T_NB):
                    cur_t = cur1_subs[tr // SUB][:, tr % SUB]  # [P, HC1, 32]
                    negz = negzp.tile([P, HC1, 32], F32, tag="negz",
                                      name="negz1")
                    nc.vector.scalar_tensor_tensor(
                        negz[:], mem1_cur[:], THR, cur_t,
                        ALU.is_gt, ALU.subtract)
                    mem1_new = statep.tile([P, HC1, 32], F32, tag="mem1",
                                           name="mem1")
                    nc.vector.scalar_tensor_tensor(
                        mem1_new[:], mem1_cur[:], BETA, negz[:],
                        ALU.mult, ALU.subtract)
                    mem1_cur = mem1_new
                    # spike of step t thresholds the POST-update membrane
                    nc.vector.tensor_scalar(
                        spk1[:, :, tr * 32:(tr + 1) * 32], mem1_cur[:],
                        THR, None, ALU.is_gt)
                return spk1

            # ---------------- main t-block pipeline -------------------------
            # software pipelining: mm1(nb+1) AND scan1(nb+1) are emitted
            # BEFORE mm2(nb)/scan2(nb).  PE order: mm1(0) mm1(1) mm2(0)
            # mm1(2) mm2(1) ... (program order = priority), and the DVE FIFO
            # runs scan1(nb+1) before scan2(nb), so mm2(nb+1) never waits on
            # a scan chain (spk1 is double-buffered to allow this overlap).
            cur1_next = x_and_mm1(0)
            spk1_cur = scan1_block(0, cur1_next)
            for nb in range(NNB):
                t0 = nb * T_NB
                if nb + 1 < NNB:
                    cur1_next = x_and_mm1(nb + 1)
                    spk1_next = scan1_block(nb + 1, cur1_next)
                spk1 = spk1_cur

                # -- matmul2 (2x bf16): cur2[(t,mc,b)] = W2 @ spk1^T + b2 ----
                ph = _Ph()
                pe_phases.append(ph)
                cur2_subs = [curp.tile([P, SUB, HC2, 32], F32, tag="cur2",
                                       bufs=5, name="cur2")
                             for _ in range(NSUB)]
                for mq in range(HC2 // MCQ):
                    pss = [mpsum.tile([P, NB32], F32, tag="mm", name="mm2ps")
                           for _ in range(MCQ)]
                    for kc in range(HC1):
                        wt = w2tp.tile([P, 2, MCQ * P], BF16, tag="w2t",
                                       name="w2t")
                        dq = nc.sync if kc % 2 == 0 else nc.scalar
                        dq.dma_start(
                            wt[:],
                            w2t_d.ap()[kc * P:(kc + 1) * P, :,
                                       mq * MCQ * P:(mq + 1) * MCQ * P])
                        rhs = spk1[:, kc, :]
                        for i in range(MCQ):
                            ph.add(nc.tensor.matmul(
                                pss[i][:], wt[:, 0, i * P:(i + 1) * P], rhs,
                                start=(kc == 0), stop=False))
                            ph.add(nc.tensor.matmul(
                                pss[i][:], wt[:, 1, i * P:(i + 1) * P], rhs,
                                start=False, stop=(kc == HC1 - 1)))
                    for i in range(MCQ):
                        mc = mq * MCQ + i
                        psv = pss[i].rearrange("p (t b) -> p t b", b=32)
                        for s in range(NSUB):
                            nc.scalar.activation(
                                cur2_subs[s][:, :, mc, :],
                                psv[:, s * SUB:(s + 1) * SUB, :],
                                AF.Identity, bias=b2s[:, mc:mc + 1])

                # -- scan2 (T_NB steps) --------------------------------------
                for tr in range(T_NB):
                    t = t0 + tr
                    cur_t = cur2_subs[tr // SUB][:, tr % SUB]
                    negz = negzp.tile([P, HC2, 32], F32, tag="negz",
                                      name="negz2")
                    nc.vector.scalar_tensor_tensor(
                        negz[:], mem2_cur[:], THR, cur_t,
                        ALU.is_gt, ALU.subtract)
                    mem2_new = statep.tile([P, HC2, 32], F32, tag="mem2",
                                           name="mem2")
                    nc.vector.scalar_tensor_tensor(
                        mem2_new[:], mem2_cur[:], BETA, negz[:],
                        ALU.mult, ALU.subtract)
                    mem2_cur = mem2_new
                    if t == T - 1:
                        nc.vector.tensor_scalar(
                            spk2_fin[:], mem2_cur[:], THR, None, ALU.is_gt)

                if nb + 1 < NNB:
                    spk1_cur = spk1_next

            # ---------------- remaining outputs -----------------------------
            emit_out(mem1_cur, HC1, mem1_d)
            emit_out(mem2_cur, HC2, mem2_d)
            emit_out(spk2_fin, HC2, spk2_d)

            # chain consecutive PE phases: every inst of phase b ordered
            # after the last inst of phase a (order-only deps)
            for a, b in zip(pe_phases, pe_phases[1:]):
                if a.insts and b.insts:
                    for bi in b.insts:
                        add_dep_helper(bi, a.insts[-1], sync=False,
                                       reason="PE phase ordering")

    nc.compile()
    return nc


_NC_CACHE = {}


def _get_nc():
    if "full" not in _NC_CACHE:
        _NC_CACHE["full"] = build_snn()
    return _NC_CACHE["full"]


def _hl_pair(a, scale_low=None, dtype=np.float16):
    """Split fp32 array into (high, low) pairs stacked on axis 1.
    With scale_low, the low term is multiplied by it (keeps tiny lows out
    of the fp16 subnormal floor; de-scaled via the moving operand)."""
    h = a.astype(dtype)
    low = a - h.astype(np.float32)
    if scale_low:
        low = low * np.float32(scale_low)
    return np.ascontiguousarray(np.stack([h, low.astype(dtype)], axis=1))


def prep_inputs(x, W1, b1, W2, b2):
    """Host-side prep: shard x over cores (transposed to [d, (t,b)]),
    transpose W1/W2, split everything into fp16 (high, low) pairs."""
    x = np.asarray(x, np.float32)
    W1 = np.asarray(W1, np.float32)
    b1 = np.ascontiguousarray(np.asarray(b1, np.float32))
    W2 = np.asarray(W2, np.float32)
    b2 = np.ascontiguousarray(np.asarray(b2, np.float32))
    B, T, D = x.shape

    W1Thl = _hl_pair(W1.T, scale_low=2048.0)            # [D, 2, H1]
    W2Thl = _hl_pair(W2.T, dtype=ml_dtypes.bfloat16)    # [H1, 2, H2]

    bl = B // N_CORES
    in_maps = []
    for c in range(N_CORES):
        xc = x[c * bl:(c + 1) * bl]                     # [bl, T, D]
        xT = np.ascontiguousarray(
            xc.transpose(2, 1, 0).reshape(D, T * bl))   # [d, (t,b)] t-major
        xh = xT.astype(np.float16)
        xds = (xh.astype(np.float32) * np.float32(1.0 / 2048.0)).astype(
            np.float16)
        xl = (xT - xh.astype(np.float32)).astype(np.float16)
        xT3 = np.ascontiguousarray(np.stack([xh, xds, xl], axis=1))
        in_maps.append({
            "xT3": xT3, "W1Thl": W1Thl, "b1": b1,
            "W2Thl": W2Thl, "b2": b2,
        })
    return in_maps


def kernel(x, W1, b1, W2, b2):
    """Full-input entry point: shards B across 8 NeuronCores, returns full
    (spk2, mem1, mem2) exactly like reference()."""
    nc = _get_nc()
    in_maps = prep_inputs(x, W1, b1, W2, b2)
    res = run_bass_kernel_spmd(nc, in_maps, core_ids=list(range(N_CORES)))
    spk2 = np.concatenate([res.results[c]["spk2"] for c in range(N_CORES)], 0)
    mem1 = np.concatenate([res.results[c]["mem1"] for c in range(N_CORES)], 0)
    mem2 = np.concatenate([res.results[c]["mem2"] for c in range(N_CORES)], 0)
    return spk2, mem1, mem2


# revision 24
# speedup vs baseline: 1.2537x; 1.0064x over previous
"""Feedforward SNN (Linear -> LIF) x2 kernel for Trainium2, 8-core data parallel.

Per-core plan (B sharded 8 ways, BL=32 samples/core):
  - Host pre-transposes operands once (cheap numpy): xT[d, (t,b)] per core,
    W1T[d, h1], W2T[h1, h2], all split Dekker-style into fp16 (high, low)
    pairs stored interleaved. The device runs ONLY matmuls and LIF scans.
  - The PE datapath is natively FP22 (11-bit mantissa): fp16 operands upcast
    exactly, so a (high, low) fp16 pair carries ~21 bits. Layer-1 currents
    need full fp32-class precision (spike-threshold crossings cascade hard),
    so mm1 = 3 cross terms (xh*wh + xl*wh + xh*wl), residual ~2^-21.
  - Layer-2's moving operand is the {0,1} spike tensor (exact in fp16);
    mm2 = 2 terms (W2h + W2l) @ spk, residual ~2^-22.
  - All matmul passes run at 1 cycle/row (fp16 = bf16 speed), weight loads
    hidden by fast-weight-load + background weight buffer.
  - Layer-1 currents for ALL timesteps of a t-block: Cur1[h1, (t,b)] =
    W1 @ x^T (x does not depend on recurrent state).
  - LIF scans over t on [128, HC*32] tiles (partition = h % 128, free =
    (hchunk, b)); fused scalar_tensor_tensor DVE ops.
  - Software-pipelined: mm1(nb+1) is emitted before mm2(nb) so the PE fills
    the scan1(nb) latency; PE phases are chained with order-only deps.
"""

import os
import sys

import numpy as np

for _p in ("/opt/trn_rl_repo", "/root/.axon_site/_ro/trn_rl_repo"):
    if os.path.isdir(_p) and _p not in sys.path:
        sys.path.insert(0, _p)

import ml_dtypes  # noqa: E402

import concourse.bass as bass  # noqa: E402
import concourse.mybir as mybir  # noqa: E402
import concourse.tile as tile  # noqa: E402
from concourse import bacc  # noqa: E402
from concourse.bass_utils import run_bass_kernel_spmd  # noqa: E402
from concourse.masks import make_identity  # noqa: E402
from concourse.tile_rust import add_dep_helper  # noqa: E402

F32 = mybir.dt.float32
F16 = mybir.dt.float16
BF16 = mybir.dt.bfloat16
ALU = mybir.AluOpType
AF = mybir.ActivationFunctionType

BETA = 0.9
THR = 1.0

B_FULL, T_FULL, D_FULL, H1_FULL, H2_FULL = 256, 64, 1024, 2048, 2048
N_CORES = 8
BL = B_FULL // N_CORES  # 32


def build_snn(T=T_FULL, D=D_FULL, H1=H1_FULL, H2=H2_FULL, T_NB=16):
    """Build the single-core Bass program (identical across the 8 cores)."""
    P = 128
    KC1 = D // P
    HC1 = H1 // P
    HC2 = H2 // P
    NNB = T // T_NB
    SUB = min(4, T_NB)
    NSUB = T_NB // SUB
    MCQ = min(4, HC2)
    HCQ = min(4, HC1)
    NB32 = T_NB * 32          # matmul free dim per t-block

    assert T % T_NB == 0 and T_NB % SUB == 0
    assert HC2 % MCQ == 0 and HC1 % HCQ == 0

    nc = bacc.Bacc("TRN2", target_bir_lowering=False, debug=False)

    # x variants: 0 = xh, 1 = xh * 2^-11 (de-scales W1's scaled low term),
    # 2 = xl.  W1 pair: (wh, wl * 2^11) -- the low is host-scaled back into
    # fp16 normal range (raw wl ~1e-5 would hit the subnormal floor).
    xt_d = nc.dram_tensor("xT3", [D, 3, T * BL], F16, kind="ExternalInput")
    w1t_d = nc.dram_tensor("W1Thl", [D, 2, H1], F16, kind="ExternalInput")
    b1_d = nc.dram_tensor("b1", [H1], F32, kind="ExternalInput")
    w2t_d = nc.dram_tensor("W2Thl", [H1, 2, H2], BF16,
                           kind="ExternalInput")
    b2_d = nc.dram_tensor("b2", [H2], F32, kind="ExternalInput")

    spk2_d = nc.dram_tensor("spk2", [BL, H2], F32, kind="ExternalOutput")
    mem1_d = nc.dram_tensor("mem1", [BL, H1], F32, kind="ExternalOutput")
    mem2_d = nc.dram_tensor("mem2", [BL, H2], F32, kind="ExternalOutput")

    with tile.TileContext(nc) as tc:
        from contextlib import ExitStack
        ctx = ExitStack()
        with ctx:
            const = ctx.enter_context(tc.tile_pool(name="const", bufs=1))
            xtp = ctx.enter_context(tc.tile_pool(name="xtp", bufs=2))
            w1tp = ctx.enter_context(tc.tile_pool(name="w1tp", bufs=6))
            w2tp = ctx.enter_context(tc.tile_pool(name="w2tp", bufs=4))
            curp = ctx.enter_context(tc.tile_pool(name="curp", bufs=6))
            spk1p = ctx.enter_context(tc.tile_pool(name="spk1p", bufs=2))
            statep = ctx.enter_context(tc.tile_pool(name="statep", bufs=2))
            negzp = ctx.enter_context(tc.tile_pool(name="negzp", bufs=1))
            outp = ctx.enter_context(tc.tile_pool(name="outp", bufs=2))
            tpsum = ctx.enter_context(
                tc.tile_pool(name="tpsum", bufs=2, space="PSUM"))
            mpsum = ctx.enter_context(
                tc.tile_pool(name="mpsum", bufs=6, space="PSUM"))

            ident = const.tile([P, P], F32, name="ident")
            make_identity(nc, ident)

            # PE phase chaining (order-only deps): keeps the mm phases
            # from interleaving in the PE stream.
            pe_phases = []

            class _Ph:
                def __init__(self):
                    self.insts = []

                def add(self, bi):
                    self.insts.append(bi.ins)

            b1s = const.tile([P, HC1], F32, name="b1s")
            b2s = const.tile([P, HC2], F32, name="b2s")

            # ---------------- PE warmup (HAM ramp) --------------------------
            wub = const.tile([P, 512], BF16, name="wub")
            nc.vector.memset(wub[:], 0.0)
            wuw = const.tile([P, P], BF16, name="wuw")
            nc.vector.memset(wuw[:], 0.0)
            ph = _Ph()
            pe_phases.append(ph)
            wups = mpsum.tile([P, 512], F32, tag="mm", name="wups")
            for i in range(18):
                ph.add(nc.tensor.matmul(wups[:], wuw[:], wub[:],
                                        start=(i == 0), stop=(i == 17)))

            # ---------------- initial LIF state ----------------------------
            mem1_cur = statep.tile([P, HC1, 32], F32, tag="mem1",
                                   name="mem1_0")
            nc.vector.memset(mem1_cur[:], 0.0)
            mem2_cur = statep.tile([P, HC2, 32], F32, tag="mem2",
                                   name="mem2_0")
            nc.vector.memset(mem2_cur[:], 0.0)
            spk2_fin = const.tile([P, HC2, 32], F32, name="spk2_fin")

            # ---------------- outputs helper --------------------------------
            def emit_out(state, nch, out_d):
                # transpose 4 h-chunks at once: [128, (4hc, 32b)] -> psum
                # [(4hc, 32b), 128], then one DMA scatters to the dram rows
                # via a (hc b) h -> b (hc h) access pattern.
                ph = _Ph()
                pe_phases.append(ph)
                for hq in range(nch // 4):
                    ps = tpsum.tile([P, P], F32, tag="tp", name="ops")
                    ph.add(nc.tensor.transpose(
                        ps[:], state[:, hq * 4:(hq + 1) * 4, :], ident[:]))
                    sb = outp.tile([P, P], F32, tag="osb", name="osb")
                    nc.scalar.activation(sb[:], ps[:], AF.Copy)
                    # psum rows are (hc, b); walk the dram side in the same
                    # (hc, b, h) order (sizes may differ from src dims --
                    # dma only requires equal totals)
                    nc.sync.dma_start(
                        out_d.ap()[:, hq * 512:(hq + 1) * 512].rearrange(
                            "b (hc h) -> hc b h", hc=4), sb[:])

            # ---------------- per-block emitters ----------------------------
            def x_and_mm1(nb):
                """xT load + matmul1 (3-term fp16 Dekker) for block nb."""
                ph = _Ph()
                pe_phases.append(ph)
                t0 = nb * T_NB
                xq = nc.gpsimd
                xt = xtp.tile([P, KC1, 3, NB32], F16, tag="xt", name="xt")
                for kc in range(KC1):
                    xq.dma_start(
                        xt[:, kc, :, :],
                        xt_d.ap()[kc * P:(kc + 1) * P, :,
                                  t0 * 32:(t0 + T_NB) * 32])
                if nb == 0:
                    # bias gathers ride gpsimd behind block 0's x tiles so
                    # they never delay the first weight/x arrivals
                    nc.gpsimd.dma_start(
                        b1s[:], b1_d.ap().rearrange("(c p) -> p c", p=P))
                    nc.gpsimd.dma_start(
                        b2s[:], b2_d.ap().rearrange("(c p) -> p c", p=P))

                cur1_subs = [curp.tile([P, SUB, HC1, 32], F32, tag="cur1",
                                       bufs=6, name="cur1")
                             for _ in range(NSUB)]
                for hq in range(HC1 // HCQ):
                    pss = [mpsum.tile([P, NB32], F32, tag="mm", name="mm1ps")
                           for _ in range(HCQ)]
                    for kc in range(KC1):
                        w1tt = w1tp.tile([P, 2, HCQ * P], F16, tag="w1t",
                                         name="w1tt")
                        # W1 rides gpsimd (with x) so mm1's first tiles are
                        # never queued behind the previous block's W2 stream
                        # on sync/scalar
                        nc.gpsimd.dma_start(
                            w1tt[:],
                            w1t_d.ap()[kc * P:(kc + 1) * P, :,
                                       hq * HCQ * P:(hq + 1) * HCQ * P])
                        xh = xt[:, kc, 0, :]
                        xds = xt[:, kc, 1, :]
                        xl = xt[:, kc, 2, :]
                        for i in range(HCQ):
                            wh = w1tt[:, 0, i * P:(i + 1) * P]
                            wl = w1tt[:, 1, i * P:(i + 1) * P]
                            ph.add(nc.tensor.matmul(
                                pss[i][:], wh, xh,
                                start=(kc == 0), stop=False))
                            ph.add(nc.tensor.matmul(
                                pss[i][:], wl, xds,
                                start=False, stop=False))
                            ph.add(nc.tensor.matmul(
                                pss[i][:], wh, xl,
                                start=False, stop=(kc == KC1 - 1)))
                    for i in range(HCQ):
                        hc = hq * HCQ + i
                        psv = pss[i].rearrange("p (t b) -> p t b", b=32)
                        for s in range(NSUB):
                            nc.scalar.activation(
                                cur1_subs[s][:, :, hc, :],
                                psv[:, s * SUB:(s + 1) * SUB, :],
                                AF.Identity, bias=b1s[:, hc:hc + 1])
                return cur1_subs

            # ---------------- scan1 emitter ---------------------------------
            def scan1_block(nb, cur1_subs):
                """LIF-1 scan for block nb -> spk1 tile (bf16)."""
                nonlocal mem1_cur
                spk1 = spk1p.tile([P, HC1, NB32], BF16, tag="spk1",
                                  name="spk1")
                for tr in range(T_NB):
                    cur_t = cur1_subs[tr // SUB][:, tr % SUB]  # [P, HC1, 32]
                    negz = negzp.tile([P, HC1, 32], F32, tag="negz",
                                      name="negz1")
                    nc.vector.scalar_tensor_tensor(
                        negz[:], mem1_cur[:], THR, cur_t,
                        ALU.is_gt, ALU.subtract)
                    mem1_new = statep.tile([P, HC1, 32], F32, tag="mem1",
                                           name="mem1")
                    nc.vector.scalar_tensor_tensor(
                        mem1_new[:], mem1_cur[:], BETA, negz[:],
                        ALU.mult, ALU.subtract)
                    mem1_cur = mem1_new
                    # spike of step t thresholds the POST-update membrane
                    nc.vector.tensor_scalar(
                        spk1[:, :, tr * 32:(tr + 1) * 32], mem1_cur[:],
                        THR, None, ALU.is_gt)
                return spk1

            # ---------------- main t-block pipeline -------------------------
            # software pipelining: mm1(nb+1) AND scan1(nb+1) are emitted
            # BEFORE mm2(nb)/scan2(nb).  PE order: mm1(0) mm1(1) mm2(0)
            # mm1(2) mm2(1) ... (program order = priority), and the DVE FIFO
            # runs scan1(nb+1) before scan2(nb), so mm2(nb+1) never waits on
            # a scan chain (spk1 is double-buffered to allow this overlap).
            cur1_next = x_and_mm1(0)
            spk1_cur = scan1_block(0, cur1_next)
            for nb in range(NNB):
                t0 = nb * T_NB
                if nb + 1 < NNB:
                    cur1_next = x_and_mm1(nb + 1)
                    spk1_next = scan1_block(nb + 1, cur1_next)
                spk1 = spk1_cur

                # -- matmul2 (2x bf16): cur2[(t,mc,b)] = W2 @ spk1^T + b2 ----
                ph = _Ph()
                pe_phases.append(ph)
                cur2_subs = [curp.tile([P, SUB, HC2, 32], F32, tag="cur2",
                                       bufs=5, name="cur2")
                             for _ in range(NSUB)]
                for mq in range(HC2 // MCQ):
                    pss = [mpsum.tile([P, NB32], F32, tag="mm", name="mm2ps")
                           for _ in range(MCQ)]
                    for kc in range(HC1):
                        wt = w2tp.tile([P, 2, MCQ * P], BF16, tag="w2t",
                                       name="w2t")
                        dq = nc.sync if kc % 2 == 0 else nc.scalar
                        dq.dma_start(
                            wt[:],
                            w2t_d.ap()[kc * P:(kc + 1) * P, :,
                                       mq * MCQ * P:(mq + 1) * MCQ * P])
                        rhs = spk1[:, kc, :]
                        for i in range(MCQ):
                            ph.add(nc.tensor.matmul(
                                pss[i][:], wt[:, 0, i * P:(i + 1) * P], rhs,
                                start=(kc == 0), stop=False))
                            ph.add(nc.tensor.matmul(
                                pss[i][:], wt[:, 1, i * P:(i + 1) * P], rhs,
                                start=False, stop=(kc == HC1 - 1)))
                    for i in range(MCQ):
                        mc = mq * MCQ + i
                        psv = pss[i].rearrange("p (t b) -> p t b", b=32)
                        for s in range(NSUB):
                            nc.scalar.activation(
                                cur2_subs[s][:, :, mc, :],
                                psv[:, s * SUB:(s + 1) * SUB, :],
                                AF.Identity, bias=b2s[:, mc:mc + 1])

                # -- scan2 (T_NB steps) --------------------------------------
                for tr in range(T_NB):
                    t = t0 + tr
                    cur_t = cur2_subs[tr // SUB][:, tr % SUB]
                    negz = negzp.tile([P, HC2, 32], F32, tag="negz",
                                      name="negz2")
                    nc.vector.scalar_tensor_tensor(
                        negz[:], mem2_cur[:], THR, cur_t,
                        ALU.is_gt, ALU.subtract)
                    mem2_new = statep.tile([P, HC2, 32], F32, tag="mem2",
                                           name="mem2")
                    nc.vector.scalar_tensor_tensor(
                        mem2_new[:], mem2_cur[:], BETA, negz[:],
                        ALU.mult, ALU.subtract)
                    mem2_cur = mem2_new
                    if t == T - 1:
                        nc.vector.tensor_scalar(
                            spk2_fin[:], mem2_cur[:], THR, None, ALU.is_gt)

                if nb + 1 < NNB:
                    spk1_cur = spk1_next

            # ---------------- remaining outputs -----------------------------
            emit_out(mem1_cur, HC1, mem1_d)
            emit_out(mem2_cur, HC2, mem2_d)
            emit_out(spk2_fin, HC2, spk2_d)

            # chain consecutive PE phases: every inst of phase b ordered
            # after the last inst of phase a (order-only deps)
            for a, b in zip(pe_phases, pe_phases[1:]):
                if a.insts and b.insts:
                    for bi in b.insts:
                        add_dep_helper(bi, a.insts[-1], sync=False,
                                       reason="PE phase ordering")

    nc.compile()
    return nc


_NC_CACHE = {}


def _get_nc():
    if "full" not in _NC_CACHE:
        _NC_CACHE["full"] = build_snn()
    return _NC_CACHE["full"]


def _hl_pair(a, scale_low=None, dtype=np.float16):
    """Split fp32 array into (high, low) pairs stacked on axis 1.
    With scale_low, the low term is multiplied by it (keeps tiny lows out
    of the fp16 subnormal floor; de-scaled via the moving operand)."""
    h = a.astype(dtype)
    low = a - h.astype(np.float32)
    if scale_low:
        low = low * np.float32(scale_low)
    return np.ascontiguousarray(np.stack([h, low.astype(dtype)], axis=1))


def prep_inputs(x, W1, b1, W2, b2):
    """Host-side prep: shard x over cores (transposed to [d, (t,b)]),
    transpose W1/W2, split everything into fp16 (high, low) pairs."""
    x = np.asarray(x, np.float32)
    W1 = np.asarray(W1, np.float32)
    b1 = np.ascontiguousarray(np.asarray(b1, np.float32))
    W2 = np.asarray(W2, np.float32)
    b2 = np.ascontiguousarray(np.asarray(b2, np.float32))
    B, T, D = x.shape

    W1Thl = _hl_pair(W1.T, scale_low=2048.0)            # [D, 2, H1]
    W2Thl = _hl_pair(W2.T, dtype=ml_dtypes.bfloat16)    # [H1, 2, H2]

    bl = B // N_CORES
    in_maps = []
    for c in range(N_CORES):
        xc = x[c * bl:(c + 1) * bl]                     # [bl, T, D]
        xT = np.ascontiguousarray(
            xc.transpose(2, 1, 0).reshape(D, T * bl))   # [d, (t,b)] t-major
        xh = xT.astype(np.float16)
        xds = (xh.astype(np.float32) * np.float32(1.0 / 2048.0)).astype(
            np.float16)
        xl = (xT - xh.astype(np.float32)).astype(np.float16)
        xT3 = np.ascontiguousarray(np.stack([xh, xds, xl], axis=1))
        in_maps.append({
            "xT3": xT3, "W1Thl": W1Thl, "b1": b1,
            "W2Thl": W2Thl, "b2": b2,
        })
    return in_maps


def kernel(x, W1, b1, W2, b2):
    """Full-input entry point: shards B across 8 NeuronCores, returns full
    (spk2, mem1, mem2) exactly like reference()."""
    nc = _get_nc()
    in_maps = prep_inputs(x, W1, b1, W2, b2)
    res = run_bass_kernel_spmd(nc, in_maps, core_ids=list(range(N_CORES)))
    spk2 = np.concatenate([res.results[c]["spk2"] for c in range(N_CORES)], 0)
    mem1 = np.concatenate([res.results[c]["mem1"] for c in range(N_CORES)], 0)
    mem2 = np.concatenate([res.results[c]["mem2"] for c in range(N_CORES)], 0)
    return spk2, mem1, mem2


# revision 26
# speedup vs baseline: 1.2697x; 1.0128x over previous
"""Feedforward SNN (Linear -> LIF) x2 kernel for Trainium2, 8-core data parallel.

Per-core plan (B sharded 8 ways, BL=32 samples/core):
  - Host pre-transposes operands once (cheap numpy): xT[d, (t,b)] per core,
    W1T[d, h1], W2T[h1, h2], split Dekker-style into 16-bit (high, low)
    pairs. The device runs ONLY matmuls and LIF scans.
  - The PE datapath is natively FP22 (11-bit mantissa); 16-bit operands
    upcast exactly, so a (high, low) pair carries 21+ bits. Layer-1 currents
    need fp32-class precision (spike-threshold crossings cascade hard), so
    mm1 = 3 fp16 cross terms (xh*wh + (wl*2^11)*(xh*2^-11) + xl*wh); the W1
    low term is host-scaled by 2^11 into fp16 normal range (raw lows ~1e-5
    would quantize at the fp16 subnormal floor) and de-scaled through a
    pre-scaled copy of the moving operand, keeping a single PSUM scale.
    Residual ~2^-21 -> 0 flipped spikes vs the fp32 reference.
  - Layer-2's moving operand is the {0,1} spike tensor (exact in bf16);
    mm2 = 2 bf16 terms (W2h + W2l) @ spk.
  - Every matmul pass streams 512 free-dim cols in ~216ns (512 cyc @2.4GHz
    + NX); 3+2 passes x 512/1024 chunk-units = the PE floor (~780us).
  - LIF scans over t on [128, HC*32] tiles (partition = h % 128, free =
    (hchunk, b)); fused scalar_tensor_tensor DVE ops.
  - Software pipelining: mm1(nb+1) AND scan1(nb+1) are emitted before
    mm2(nb)/scan2(nb): the PE never waits on a scan (spk1 double-buffered),
    and the DVE FIFO runs scan1(nb+1) ahead of scan2(nb). PE phases are
    chained with order-only deps. Outputs are transposed 4 h-chunks per PE
    transpose and scattered with a permuted dram access pattern.
  - DMA: x + W1 ride the gpsimd queue (W2 owns sync/scalar), except block
    0's W1 which uses the then-idle sync/scalar pair.
"""

import os
import sys

import numpy as np

for _p in ("/opt/trn_rl_repo", "/root/.axon_site/_ro/trn_rl_repo"):
    if os.path.isdir(_p) and _p not in sys.path:
        sys.path.insert(0, _p)

import ml_dtypes  # noqa: E402

import concourse.bass as bass  # noqa: E402
import concourse.mybir as mybir  # noqa: E402
import concourse.tile as tile  # noqa: E402
from concourse import bacc  # noqa: E402
from concourse.bass_utils import run_bass_kernel_spmd  # noqa: E402
from concourse.masks import make_identity  # noqa: E402
from concourse.tile_rust import add_dep_helper  # noqa: E402

F32 = mybir.dt.float32
F16 = mybir.dt.float16
BF16 = mybir.dt.bfloat16
ALU = mybir.AluOpType
AF = mybir.ActivationFunctionType

BETA = 0.9
THR = 1.0

B_FULL, T_FULL, D_FULL, H1_FULL, H2_FULL = 256, 64, 1024, 2048, 2048
N_CORES = 8
BL = B_FULL // N_CORES  # 32


def build_snn(T=T_FULL, D=D_FULL, H1=H1_FULL, H2=H2_FULL, T_NB=16):
    """Build the single-core Bass program (identical across the 8 cores)."""
    P = 128
    KC1 = D // P
    HC1 = H1 // P
    HC2 = H2 // P
    NNB = T // T_NB
    SUB = min(4, T_NB)
    NSUB = T_NB // SUB
    MCQ = min(4, HC2)
    HCQ = min(4, HC1)
    NB32 = T_NB * 32          # matmul free dim per t-block

    assert T % T_NB == 0 and T_NB % SUB == 0
    assert HC2 % MCQ == 0 and HC1 % HCQ == 0

    nc = bacc.Bacc("TRN2", target_bir_lowering=False, debug=False)

    # x variants: 0 = xh, 1 = xh * 2^-11 (de-scales W1's scaled low term),
    # 2 = xl.  W1 pair: (wh, wl * 2^11) -- the low is host-scaled back into
    # fp16 normal range (raw wl ~1e-5 would hit the subnormal floor).
    xt_d = nc.dram_tensor("xT3", [D, 3, T * BL], F16, kind="ExternalInput")
    w1t_d = nc.dram_tensor("W1Thl", [D, 2, H1], F16, kind="ExternalInput")
    b1_d = nc.dram_tensor("b1", [H1], F32, kind="ExternalInput")
    w2t_d = nc.dram_tensor("W2Thl", [H1, 2, H2], BF16,
                           kind="ExternalInput")
    b2_d = nc.dram_tensor("b2", [H2], F32, kind="ExternalInput")

    spk2_d = nc.dram_tensor("spk2", [BL, H2], F32, kind="ExternalOutput")
    mem1_d = nc.dram_tensor("mem1", [BL, H1], F32, kind="ExternalOutput")
    mem2_d = nc.dram_tensor("mem2", [BL, H2], F32, kind="ExternalOutput")

    with tile.TileContext(nc) as tc:
        from contextlib import ExitStack
        ctx = ExitStack()
        with ctx:
            const = ctx.enter_context(tc.tile_pool(name="const", bufs=1))
            xtp = ctx.enter_context(tc.tile_pool(name="xtp", bufs=2))
            w1tp = ctx.enter_context(tc.tile_pool(name="w1tp", bufs=6))
            w2tp = ctx.enter_context(tc.tile_pool(name="w2tp", bufs=4))
            curp = ctx.enter_context(tc.tile_pool(name="curp", bufs=6))
            spk1p = ctx.enter_context(tc.tile_pool(name="spk1p", bufs=2))
            statep = ctx.enter_context(tc.tile_pool(name="statep", bufs=2))
            negzp = ctx.enter_context(tc.tile_pool(name="negzp", bufs=1))
            outp = ctx.enter_context(tc.tile_pool(name="outp", bufs=2))
            tpsum = ctx.enter_context(
                tc.tile_pool(name="tpsum", bufs=2, space="PSUM"))
            mpsum = ctx.enter_context(
                tc.tile_pool(name="mpsum", bufs=6, space="PSUM"))

            ident = const.tile([P, P], F32, name="ident")
            make_identity(nc, ident)

            # PE phase chaining (order-only deps): keeps the mm phases
            # from interleaving in the PE stream.
            pe_phases = []

            class _Ph:
                def __init__(self):
                    self.insts = []

                def add(self, bi):
                    self.insts.append(bi.ins)

            b1s = const.tile([P, HC1], F32, name="b1s")
            b2s = const.tile([P, HC2], F32, name="b2s")

            # ---------------- PE warmup (HAM ramp) --------------------------
            wub = const.tile([P, 512], BF16, name="wub")
            nc.vector.memset(wub[:], 0.0)
            wuw = const.tile([P, P], BF16, name="wuw")
            nc.vector.memset(wuw[:], 0.0)
            ph = _Ph()
            pe_phases.append(ph)
            wups = mpsum.tile([P, 512], F32, tag="mm", name="wups")
            for i in range(18):
                ph.add(nc.tensor.matmul(wups[:], wuw[:], wub[:],
                                        start=(i == 0), stop=(i == 17)))

            # ---------------- initial LIF state ----------------------------
            mem1_cur = statep.tile([P, HC1, 32], F32, tag="mem1",
                                   name="mem1_0")
            nc.vector.memset(mem1_cur[:], 0.0)
            mem2_cur = statep.tile([P, HC2, 32], F32, tag="mem2",
                                   name="mem2_0")
            nc.vector.memset(mem2_cur[:], 0.0)
            spk2_fin = const.tile([P, HC2, 32], F32, name="spk2_fin")

            # ---------------- outputs helper --------------------------------
            def emit_out(state, nch, out_d):
                # transpose 4 h-chunks at once: [128, (4hc, 32b)] -> psum
                # [(4hc, 32b), 128], then one DMA scatters to the dram rows
                # via a (hc b) h -> b (hc h) access pattern.
                ph = _Ph()
                pe_phases.append(ph)
                for hq in range(nch // 4):
                    ps = tpsum.tile([P, P], F32, tag="tp", name="ops")
                    ph.add(nc.tensor.transpose(
                        ps[:], state[:, hq * 4:(hq + 1) * 4, :], ident[:]))
                    sb = outp.tile([P, P], F32, tag="osb", name="osb")
                    nc.scalar.activation(sb[:], ps[:], AF.Copy)
                    # psum rows are (hc, b); walk the dram side in the same
                    # (hc, b, h) order (sizes may differ from src dims --
                    # dma only requires equal totals)
                    nc.sync.dma_start(
                        out_d.ap()[:, hq * 512:(hq + 1) * 512].rearrange(
                            "b (hc h) -> hc b h", hc=4), sb[:])

            # ---------------- per-block emitters ----------------------------
            def x_and_mm1(nb):
                """xT load + matmul1 (3-term fp16 Dekker) for block nb."""
                ph = _Ph()
                pe_phases.append(ph)
                t0 = nb * T_NB
                xq = nc.gpsimd
                xt = xtp.tile([P, KC1, 3, NB32], F16, tag="xt", name="xt")
                for kc in range(KC1):
                    xq.dma_start(
                        xt[:, kc, :, :],
                        xt_d.ap()[kc * P:(kc + 1) * P, :,
                                  t0 * 32:(t0 + T_NB) * 32])
                if nb == 0:
                    # bias gathers ride gpsimd behind block 0's x tiles so
                    # they never delay the first weight/x arrivals
                    nc.gpsimd.dma_start(
                        b1s[:], b1_d.ap().rearrange("(c p) -> p c", p=P))
                    nc.gpsimd.dma_start(
                        b2s[:], b2_d.ap().rearrange("(c p) -> p c", p=P))

                cur1_subs = [curp.tile([P, SUB, HC1, 32], F32, tag="cur1",
                                       bufs=6, name="cur1")
                             for _ in range(NSUB)]
                for hq in range(HC1 // HCQ):
                    pss = [mpsum.tile([P, NB32], F32, tag="mm", name="mm1ps")
                           for _ in range(HCQ)]
                    for kc in range(KC1):
                        w1tt = w1tp.tile([P, 2, HCQ * P], F16, tag="w1t",
                                         name="w1tt")
                        # W1 rides gpsimd (with x) so mm1's first tiles are
                        # never queued behind the previous block's W2 stream
                        # on sync/scalar -- except block 0, where sync/scalar
                        # are still empty and gpsimd is busy with x(0)
                        if nb == 0:
                            dq = nc.sync if kc % 2 == 0 else nc.scalar
                        else:
                            dq = nc.gpsimd
                        dq.dma_start(
                            w1tt[:],
                            w1t_d.ap()[kc * P:(kc + 1) * P, :,
                                       hq * HCQ * P:(hq + 1) * HCQ * P])
                        xh = xt[:, kc, 0, :]
                        xds = xt[:, kc, 1, :]
                        xl = xt[:, kc, 2, :]
                        for i in range(HCQ):
                            wh = w1tt[:, 0, i * P:(i + 1) * P]
                            wl = w1tt[:, 1, i * P:(i + 1) * P]
                            ph.add(nc.tensor.matmul(
                                pss[i][:], wh, xh,
                                start=(kc == 0), stop=False))
                            ph.add(nc.tensor.matmul(
                                pss[i][:], wl, xds,
                                start=False, stop=False))
                            ph.add(nc.tensor.matmul(
                                pss[i][:], wh, xl,
                                start=False, stop=(kc == KC1 - 1)))
                    for i in range(HCQ):
                        hc = hq * HCQ + i
                        psv = pss[i].rearrange("p (t b) -> p t b", b=32)
                        for s in range(NSUB):
                            nc.scalar.activation(
                                cur1_subs[s][:, :, hc, :],
                                psv[:, s * SUB:(s + 1) * SUB, :],
                                AF.Identity, bias=b1s[:, hc:hc + 1])
                return cur1_subs

            # ---------------- scan1 emitter ---------------------------------
            def scan1_block(nb, cur1_subs):
                """LIF-1 scan for block nb -> spk1 tile (bf16)."""
                nonlocal mem1_cur
                spk1 = spk1p.tile([P, HC1, NB32], BF16, tag="spk1",
                                  name="spk1")
                for tr in range(T_NB):
                    cur_t = cur1_subs[tr // SUB][:, tr % SUB]  # [P, HC1, 32]
                    negz = negzp.tile([P, HC1, 32], F32, tag="negz",
                                      name="negz1")
                    nc.vector.scalar_tensor_tensor(
                        negz[:], mem1_cur[:], THR, cur_t,
                        ALU.is_gt, ALU.subtract)
                    mem1_new = statep.tile([P, HC1, 32], F32, tag="mem1",
                                           name="mem1")
                    nc.vector.scalar_tensor_tensor(
                        mem1_new[:], mem1_cur[:], BETA, negz[:],
                        ALU.mult, ALU.subtract)
                    mem1_cur = mem1_new
                    # spike of step t thresholds the POST-update membrane
                    nc.vector.tensor_scalar(
                        spk1[:, :, tr * 32:(tr + 1) * 32], mem1_cur[:],
                        THR, None, ALU.is_gt)
                return spk1

            # ---------------- main t-block pipeline -------------------------
            # software pipelining: mm1(nb+1) AND scan1(nb+1) are emitted
            # BEFORE mm2(nb)/scan2(nb).  PE order: mm1(0) mm1(1) mm2(0)
            # mm1(2) mm2(1) ... (program order = priority), and the DVE FIFO
            # runs scan1(nb+1) before scan2(nb), so mm2(nb+1) never waits on
            # a scan chain (spk1 is double-buffered to allow this overlap).
            cur1_next = x_and_mm1(0)
            spk1_cur = scan1_block(0, cur1_next)
            for nb in range(NNB):
                t0 = nb * T_NB
                if nb + 1 < NNB:
                    cur1_next = x_and_mm1(nb + 1)
                    spk1_next = scan1_block(nb + 1, cur1_next)
                spk1 = spk1_cur

                # -- matmul2 (2x bf16): cur2[(t,mc,b)] = W2 @ spk1^T + b2 ----
                ph = _Ph()
                pe_phases.append(ph)
                cur2_subs = [curp.tile([P, SUB, HC2, 32], F32, tag="cur2",
                                       bufs=5, name="cur2")
                             for _ in range(NSUB)]
                for mq in range(HC2 // MCQ):
                    pss = [mpsum.tile([P, NB32], F32, tag="mm", name="mm2ps")
                           for _ in range(MCQ)]
                    for kc in range(HC1):
                        wt = w2tp.tile([P, 2, MCQ * P], BF16, tag="w2t",
                                       name="w2t")
                        dq = nc.sync if kc % 2 == 0 else nc.scalar
                        dq.dma_start(
                            wt[:],
                            w2t_d.ap()[kc * P:(kc + 1) * P, :,
                                       mq * MCQ * P:(mq + 1) * MCQ * P])
                        rhs = spk1[:, kc, :]
                        for i in range(MCQ):
                            ph.add(nc.tensor.matmul(
                                pss[i][:], wt[:, 0, i * P:(i + 1) * P], rhs,
                                start=(kc == 0), stop=False))
                            ph.add(nc.tensor.matmul(
                                pss[i][:], wt[:, 1, i * P:(i + 1) * P], rhs,
                                start=False, stop=(kc == HC1 - 1)))
                    for i in range(MCQ):
                        mc = mq * MCQ + i
                        psv = pss[i].rearrange("p (t b) -> p t b", b=32)
                        for s in range(NSUB):
                            nc.scalar.activation(
                                cur2_subs[s][:, :, mc, :],
                                psv[:, s * SUB:(s + 1) * SUB, :],
                                AF.Identity, bias=b2s[:, mc:mc + 1])

                # -- scan2 (T_NB steps) --------------------------------------
                for tr in range(T_NB):
                    t = t0 + tr
                    cur_t = cur2_subs[tr // SUB][:, tr % SUB]
                    negz = negzp.tile([P, HC2, 32], F32, tag="negz",
                                      name="negz2")
                    nc.vector.scalar_tensor_tensor(
                        negz[:], mem2_cur[:], THR, cur_t,
                        ALU.is_gt, ALU.subtract)
                    mem2_new = statep.tile([P, HC2, 32], F32, tag="mem2",
                                           name="mem2")
                    nc.vector.scalar_tensor_tensor(
                        mem2_new[:], mem2_cur[:], BETA, negz[:],
                        ALU.mult, ALU.subtract)
                    mem2_cur = mem2_new
                    if t == T - 1:
                        nc.vector.tensor_scalar(
                            spk2_fin[:], mem2_cur[:], THR, None, ALU.is_gt)

                if nb + 1 < NNB:
                    spk1_cur = spk1_next

            # ---------------- remaining outputs -----------------------------
            emit_out(mem1_cur, HC1, mem1_d)
            emit_out(mem2_cur, HC2, mem2_d)
            emit_out(spk2_fin, HC2, spk2_d)

            # chain consecutive PE phases: every inst of phase b ordered
            # after the last inst of phase a (order-only deps)
            for a, b in zip(pe_phases, pe_phases[1:]):
                if a.insts and b.insts:
                    for bi in b.insts:
                        add_dep_helper(bi, a.insts[-1], sync=False,
                                       reason="PE phase ordering")

    nc.compile()
    return nc


_NC_CACHE = {}


def _get_nc():
    if "full" not in _NC_CACHE:
        _NC_CACHE["full"] = build_snn()
    return _NC_CACHE["full"]


def _hl_pair(a, scale_low=None, dtype=np.float16):
    """Split fp32 array into (high, low) pairs stacked on axis 1.
    With scale_low, the low term is multiplied by it (keeps tiny lows out
    of the fp16 subnormal floor; de-scaled via the moving operand)."""
    h = a.astype(dtype)
    low = a - h.astype(np.float32)
    if scale_low:
        low = low * np.float32(scale_low)
    return np.ascontiguousarray(np.stack([h, low.astype(dtype)], axis=1))


def prep_inputs(x, W1, b1, W2, b2):
    """Host-side prep: shard x over cores (transposed to [d, (t,b)]),
    transpose W1/W2, split everything into fp16 (high, low) pairs."""
    x = np.asarray(x, np.float32)
    W1 = np.asarray(W1, np.float32)
    b1 = np.ascontiguousarray(np.asarray(b1, np.float32))
    W2 = np.asarray(W2, np.float32)
    b2 = np.ascontiguousarray(np.asarray(b2, np.float32))
    B, T, D = x.shape

    W1Thl = _hl_pair(W1.T, scale_low=2048.0)            # [D, 2, H1]
    W2Thl = _hl_pair(W2.T, dtype=ml_dtypes.bfloat16)    # [H1, 2, H2]

    bl = B // N_CORES
    in_maps = []
    for c in range(N_CORES):
        xc = x[c * bl:(c + 1) * bl]                     # [bl, T, D]
        xT = np.ascontiguousarray(
            xc.transpose(2, 1, 0).reshape(D, T * bl))   # [d, (t,b)] t-major
        xh = xT.astype(np.float16)
        xds = (xh.astype(np.float32) * np.float32(1.0 / 2048.0)).astype(
            np.float16)
        xl = (xT - xh.astype(np.float32)).astype(np.float16)
        xT3 = np.ascontiguousarray(np.stack([xh, xds, xl], axis=1))
        in_maps.append({
            "xT3": xT3, "W1Thl": W1Thl, "b1": b1,
            "W2Thl": W2Thl, "b2": b2,
        })
    return in_maps


def kernel(x, W1, b1, W2, b2):
    """Full-input entry point: shards B across 8 NeuronCores, returns full
    (spk2, mem1, mem2) exactly like reference()."""
    nc = _get_nc()
    in_maps = prep_inputs(x, W1, b1, W2, b2)
    res = run_bass_kernel_spmd(nc, in_maps, core_ids=list(range(N_CORES)))
    spk2 = np.concatenate([res.results[c]["spk2"] for c in range(N_CORES)], 0)
    mem1 = np.concatenate([res.results[c]["mem1"] for c in range(N_CORES)], 0)
    mem2 = np.concatenate([res.results[c]["mem2"] for c in range(N_CORES)], 0)
    return spk2, mem1, mem2
